# revision 13
# baseline (speedup 1.0000x reference)
"""Trainium2 Bass kernel for nn_LocalClassifier (moe_routing).

Computation (reference):
    xr     = x.reshape(B, P, F)            # [32, 784, 2048] fp32
    Wg     = W[target]                     # [32, 2048]  per-batch gathered row
    logits = einsum('bpf,bf->bp', xr, Wg) + b[target][:, None]
    out    = sigmoid(logits).reshape(-1, 1, 1, 1)    # [25088, 1, 1, 1]

Strategy (8 NeuronCores, data parallel over B):
  - Host gathers the 4 W rows / bias values each core needs (the "routing"),
    shards B across the 8 cores (4 batches -> 3136 of the 25088 rows each),
    and pre-transposes each core's x shard to feature-major fp16 layout so
    the TensorEngine contracts over features (K on partitions) with plain
    contiguous DMAs.  fp16 operands: PE streams single-pass (fp32 lowers to
    a 2x LO/HI pass) and HBM traffic halves; PSUM accumulates fp32.
  - The 4 batches map to the PE array's four 32-wide column groups
    (tile_position (0, 32*b)), so their 128x1x392 matmuls run concurrently
    instead of back-to-back.
  - Streaming: uneven chunk-groups [5,5,5,1] x 4 batches, one DMA each
    (1.2/0.25 MB), alternated across the two HWDGE rings (SP/ACT); all
    tiles resident (bufs=16) so DMA never stalls on buffer recycling.  The
    tiny last group keeps the post-stream PE tail under 1 us.
  - Epilogue: two fused bias+sigmoid activations over PSUM partitions 0-96
    (only rows {0,32,64,96} are consumed), one strided DMA writes the
    [4, 784] fp32 outputs.
  - Memory-bound: 12.8 MB/core HBM reads at ~360 GB/s -> ~36 us roofline.
"""

import sys

sys.path.insert(0, "/opt/trn_rl_repo")

import numpy as np

import concourse.bacc as bacc
import concourse.mybir as mybir
import concourse.tile as tile
from concourse.bass_utils import run_bass_kernel_spmd

B = 32      # batches
P = 784     # pixels per batch
F = 2048    # features
NCORES = 8
BPC = B // NCORES          # 4 batches per core
KC = F // 128              # 16 feature chunks of 128
GROUPS = [5, 5, 5, 1]      # chunks per DMA group (small tail group)
NH = 2                     # split P into 2 matmul halves (PSUM bank = 512 fp32)
NHALF = P // NH            # 392

FP32 = mybir.dt.float32
FP16 = mybir.dt.float16

_NC_CACHE = {}


def _build_nc():
    nc = bacc.Bacc()
    xt = nc.declare_dram_parameter("xt", [BPC, KC * 128 * P], FP16, isOutput=False)
    wg = nc.declare_dram_parameter("wg", [128, BPC * KC], FP16, isOutput=False)
    bg = nc.declare_dram_parameter("bg", [128, 1], FP32, isOutput=False)
    out = nc.declare_dram_parameter("out", [BPC, P], FP32, isOutput=True)

    with tile.TileContext(nc) as tc:
        with (
            tc.tile_pool(name="xpool", bufs=len(GROUPS) * BPC) as xpool,
            tc.tile_pool(name="cpool", bufs=1) as cpool,
            tc.tile_pool(name="psum", bufs=1, space="PSUM") as pp,
        ):
            wg_sb = cpool.tile([128, BPC * KC], FP16)
            bg_sb = cpool.tile([128, 1], FP32)
            out_sb = cpool.tile([128, P], FP32)

            # batch b accumulates in PSUM partition strip [32b, 32b+1)
            ps = [
                pp.tile([128, NHALF], FP32, name=f"ps{h}", tag=f"ps{h}")
                for h in range(NH)
            ]

            off = 0
            di = 0
            for g, n in enumerate(GROUPS):
                tiles = []
                for b in range(BPC):
                    t = xpool.tile([128, n * P], FP16, name=f"x{g}{b}", tag="xt")
                    eng = nc.sync if di % 2 == 0 else nc.scalar
                    di += 1
                    eng.dma_start(
                        out=t[:],
                        in_=xt[b, off * 128 * P : (off + n) * 128 * P].rearrange(
                            "(p f) -> p f", p=128
                        ),
                    )
                    tiles.append(t)
                if g == 0:
                    # constants issued after the first wave's tiles so they
                    # don't delay the x stream; they still land well before
                    # the first matmul needs them
                    nc.scalar.dma_start(out=wg_sb[:], in_=wg[:])
                    nc.scalar.dma_start(out=bg_sb[:], in_=bg[:])
                for c in range(n):
                    k = off + c
                    # h-major so each half's accumulation closes as early
                    # as possible in the final wave
                    for h in range(NH):
                        for b in range(BPC):
                            col = b * KC + k
                            nc.tensor.matmul(
                                ps[h][32 * b : 32 * b + 1, :],
                                wg_sb[:, col : col + 1],
                                tiles[b][
                                    :, c * P + h * NHALF : c * P + (h + 1) * NHALF
                                ],
                                start=(k == 0),
                                stop=(k == KC - 1),
                                tile_position=(0, 32 * b),
                            )
                off += n

            # one activation per half over partitions 0..96; lanes other
            # than {0,32,64,96} compute on garbage and are never read.
            # Per-half epilogue so sigmoid+store of half 0 overlap the
            # final matmuls of half 1.
            for h in range(NH):
                nc.scalar.activation(
                    out_sb[0:97, h * NHALF : (h + 1) * NHALF],
                    ps[h][0:97, :],
                    mybir.ActivationFunctionType.Sigmoid,
                    bias=bg_sb[0:97, 0:1],
                    scale=1.0,
                )
                nc.sync.dma_start(
                    out=out[:, h * NHALF : (h + 1) * NHALF],
                    in_=out_sb[0:128:32, h * NHALF : (h + 1) * NHALF],
                )

    nc.finalize()
    return nc


def _get_nc():
    if "nc" not in _NC_CACHE:
        _NC_CACHE["nc"] = _build_nc()
    return _NC_CACHE["nc"]


def _make_in_maps(x, target, W, b):
    x = np.asarray(x, dtype=np.float32).reshape(B, P, F)
    target = np.asarray(target).astype(np.int64)
    W = np.asarray(W, dtype=np.float32)
    b = np.asarray(b, dtype=np.float32)

    Wg = W[target]          # [B, F]
    bg = b[target]          # [B]

    in_maps = []
    for m in range(NCORES):
        sl = slice(m * BPC, (m + 1) * BPC)
        # (b, e, k, p) -> (b, k, p, e), fp16
        xs = (
            x[sl]
            .reshape(BPC, P, KC, 128)
            .transpose(0, 2, 3, 1)
            .astype(np.float16)
        )  # [BPC, KC, 128, P]
        # per group: (k, p, e) -> (p, k, e) so each partition's group data
        # is one contiguous run
        parts = []
        off = 0
        for n in GROUPS:
            grp = xs[:, off : off + n].transpose(0, 2, 1, 3)  # [BPC, 128, n, P]
            parts.append(grp.reshape(BPC, n * 128 * P))
            off += n
        xtc = np.ascontiguousarray(np.concatenate(parts, axis=1))
        # wg[p, b*KC + k] = Wg[b, k*128 + p]
        wgc = (
            Wg[sl]
            .reshape(BPC, KC, 128)
            .transpose(2, 0, 1)
            .reshape(128, BPC * KC)
            .astype(np.float16)
        )
        bgs = np.zeros((128, 1), np.float32)
        bgs[np.arange(BPC) * 32, 0] = bg[sl]
        in_maps.append({"xt": xtc, "wg": np.ascontiguousarray(wgc), "bg": bgs})
    return in_maps


def run(x, target, W, b, trace=False, **trace_kwargs):
    """Run on 8 cores; returns (full_output, BassKernelResults)."""
    nc = _get_nc()
    in_maps = _make_in_maps(x, target, W, b)
    res = run_bass_kernel_spmd(
        nc, in_maps, list(range(NCORES)), trace=trace, **trace_kwargs
    )
    outs = [res.results[i]["out"].reshape(-1) for i in range(NCORES)]
    full = np.concatenate(outs, axis=0).reshape(-1, 1, 1, 1).astype(np.float32)
    return full, res


def kernel(x, target, W, b):
    full, _ = run(x, target, W, b, trace=False)
    return full


# revision 14
# speedup vs baseline: 1.0907x; 1.0907x over previous
"""Trainium2 Bass kernel for nn_LocalClassifier (moe_routing).

Computation (reference):
    xr     = x.reshape(B, P, F)            # [32, 784, 2048] fp32
    Wg     = W[target]                     # [32, 2048]  per-batch gathered row
    logits = einsum('bpf,bf->bp', xr, Wg) + b[target][:, None]
    out    = sigmoid(logits).reshape(-1, 1, 1, 1)    # [25088, 1, 1, 1]

Strategy (8 NeuronCores, data parallel over B):
  - Host gathers the 4 W rows / bias values each core needs (the "routing"),
    shards B across the 8 cores (4 batches -> 3136 of the 25088 rows each),
    and pre-transposes each core's x shard to feature-major fp16 layout so
    the TensorEngine contracts over features (K on partitions) with plain
    contiguous DMAs.  fp16 operands: PE streams single-pass (fp32 lowers to
    a 2x LO/HI pass) and HBM traffic halves; PSUM accumulates fp32.
  - The 4 batches map to the PE array's four 32-wide column groups
    (tile_position (0, 32*b)), so their 128x1x392 matmuls run concurrently
    instead of back-to-back.
  - Streaming: uneven chunk-groups [5,5,5,1] x 4 batches, one DMA each
    (1.2/0.25 MB), alternated across the two HWDGE rings (SP/ACT); all
    tiles resident (bufs=16) so DMA never stalls on buffer recycling.  The
    tiny last group keeps the post-stream PE tail under 1 us.
  - Epilogue: two fused bias+sigmoid activations over PSUM partitions 0-96
    (only rows {0,32,64,96} are consumed), one strided DMA writes the
    [4, 784] fp32 outputs.
  - Memory-bound: 12.8 MB/core HBM reads at ~360 GB/s -> ~36 us roofline.
"""

import sys

sys.path.insert(0, "/opt/trn_rl_repo")

import numpy as np

import concourse.bacc as bacc
import concourse.mybir as mybir
import concourse.tile as tile
from concourse.bass_utils import run_bass_kernel_spmd

B = 32      # batches
P = 784     # pixels per batch
F = 2048    # features
NCORES = 8
BPC = B // NCORES          # 4 batches per core
KC = F // 128              # 16 feature chunks of 128
GROUPS = [8, 7, 1]         # chunks per DMA group (small tail group)
NH = 2                     # split P into 2 matmul halves (PSUM bank = 512 fp32)
NHALF = P // NH            # 392

FP32 = mybir.dt.float32
FP16 = mybir.dt.float16

_NC_CACHE = {}


def _build_nc():
    nc = bacc.Bacc()
    xt = nc.declare_dram_parameter("xt", [BPC, KC * 128 * P], FP16, isOutput=False)
    wg = nc.declare_dram_parameter("wg", [128, BPC * KC], FP16, isOutput=False)
    bg = nc.declare_dram_parameter("bg", [128, 1], FP32, isOutput=False)
    out = nc.declare_dram_parameter("out", [BPC, P], FP32, isOutput=True)

    with tile.TileContext(nc) as tc:
        with (
            tc.tile_pool(name="xpool", bufs=len(GROUPS) * BPC) as xpool,
            tc.tile_pool(name="cpool", bufs=1) as cpool,
            tc.tile_pool(name="psum", bufs=1, space="PSUM") as pp,
        ):
            wg_sb = cpool.tile([128, BPC * KC], FP16)
            bg_sb = cpool.tile([128, 1], FP32)
            out_sb = cpool.tile([128, P], FP32)

            # batch b accumulates in PSUM partition strip [32b, 32b+1)
            ps = [
                pp.tile([128, NHALF], FP32, name=f"ps{h}", tag=f"ps{h}")
                for h in range(NH)
            ]

            off = 0
            di = 0
            for g, n in enumerate(GROUPS):
                tiles = []
                for b in range(BPC):
                    t = xpool.tile([128, n * P], FP16, name=f"x{g}{b}", tag="xt")
                    eng = nc.sync if di % 2 == 0 else nc.scalar
                    di += 1
                    eng.dma_start(
                        out=t[:],
                        in_=xt[b, off * 128 * P : (off + n) * 128 * P].rearrange(
                            "(p f) -> p f", p=128
                        ),
                    )
                    tiles.append(t)
                if g == 0:
                    # constants issued after the first wave's tiles so they
                    # don't delay the x stream; they still land well before
                    # the first matmul needs them
                    nc.scalar.dma_start(out=wg_sb[:], in_=wg[:])
                    nc.scalar.dma_start(out=bg_sb[:], in_=bg[:])
                for c in range(n):
                    k = off + c
                    # h-major so each half's accumulation closes as early
                    # as possible in the final wave
                    for h in range(NH):
                        for b in range(BPC):
                            col = b * KC + k
                            nc.tensor.matmul(
                                ps[h][32 * b : 32 * b + 1, :],
                                wg_sb[:, col : col + 1],
                                tiles[b][
                                    :, c * P + h * NHALF : c * P + (h + 1) * NHALF
                                ],
                                start=(k == 0),
                                stop=(k == KC - 1),
                                tile_position=(0, 32 * b),
                            )
                off += n

            # one activation per half over partitions 0..96; lanes other
            # than {0,32,64,96} compute on garbage and are never read.
            # Per-half epilogue so sigmoid+store of half 0 overlap the
            # final matmuls of half 1.
            for h in range(NH):
                nc.scalar.activation(
                    out_sb[0:97, h * NHALF : (h + 1) * NHALF],
                    ps[h][0:97, :],
                    mybir.ActivationFunctionType.Sigmoid,
                    bias=bg_sb[0:97, 0:1],
                    scale=1.0,
                )
                nc.sync.dma_start(
                    out=out[:, h * NHALF : (h + 1) * NHALF],
                    in_=out_sb[0:128:32, h * NHALF : (h + 1) * NHALF],
                )

    nc.finalize()
    return nc


def _get_nc():
    if "nc" not in _NC_CACHE:
        _NC_CACHE["nc"] = _build_nc()
    return _NC_CACHE["nc"]


def _make_in_maps(x, target, W, b):
    x = np.asarray(x, dtype=np.float32).reshape(B, P, F)
    target = np.asarray(target).astype(np.int64)
    W = np.asarray(W, dtype=np.float32)
    b = np.asarray(b, dtype=np.float32)

    Wg = W[target]          # [B, F]
    bg = b[target]          # [B]

    in_maps = []
    for m in range(NCORES):
        sl = slice(m * BPC, (m + 1) * BPC)
        # (b, e, k, p) -> (b, k, p, e), fp16
        xs = (
            x[sl]
            .reshape(BPC, P, KC, 128)
            .transpose(0, 2, 3, 1)
            .astype(np.float16)
        )  # [BPC, KC, 128, P]
        # per group: (k, p, e) -> (p, k, e) so each partition's group data
        # is one contiguous run
        parts = []
        off = 0
        for n in GROUPS:
            grp = xs[:, off : off + n].transpose(0, 2, 1, 3)  # [BPC, 128, n, P]
            parts.append(grp.reshape(BPC, n * 128 * P))
            off += n
        xtc = np.ascontiguousarray(np.concatenate(parts, axis=1))
        # wg[p, b*KC + k] = Wg[b, k*128 + p]
        wgc = (
            Wg[sl]
            .reshape(BPC, KC, 128)
            .transpose(2, 0, 1)
            .reshape(128, BPC * KC)
            .astype(np.float16)
        )
        bgs = np.zeros((128, 1), np.float32)
        bgs[np.arange(BPC) * 32, 0] = bg[sl]
        in_maps.append({"xt": xtc, "wg": np.ascontiguousarray(wgc), "bg": bgs})
    return in_maps


def run(x, target, W, b, trace=False, **trace_kwargs):
    """Run on 8 cores; returns (full_output, BassKernelResults)."""
    nc = _get_nc()
    in_maps = _make_in_maps(x, target, W, b)
    res = run_bass_kernel_spmd(
        nc, in_maps, list(range(NCORES)), trace=trace, **trace_kwargs
    )
    outs = [res.results[i]["out"].reshape(-1) for i in range(NCORES)]
    full = np.concatenate(outs, axis=0).reshape(-1, 1, 1, 1).astype(np.float32)
    return full, res


def kernel(x, target, W, b):
    full, _ = run(x, target, W, b, trace=False)
    return full
